# revision 5
# baseline (speedup 1.0000x reference)
"""DbrxExperts MoE kernel for 8 Trainium2 NeuronCores (expert-parallel).

Problem: E=16 experts, top_k=4, H=2048, F=4096, T=64 tokens.
out = sum_e r[:, e] * (silu(x @ w1_e.T) * (x @ v1_e.T)) @ w2_e
with r = scatter-add of top_weights into dense [T, E].

Strategy: expert-parallel across 8 cores (2 experts per core). Each core
streams its 2 experts' weights (bf16-cast on host: halves HBM traffic;
fp32 PSUM accumulation keeps rel-err ~4e-3) and computes a partial
output [T, H]; host sums the 8 partials. Routing weights are folded into
the `up` projection drain, so the down-projection accumulates both local
experts directly in PSUM.

Weight layouts are pre-swizzled on the host so every weight DMA is a
fully contiguous 2 MiB transfer of shape [128, 8192] bf16.
"""

import os
import sys
import types

import numpy as np
import ml_dtypes

BF16 = ml_dtypes.bfloat16

E, TOPK, H, F = 16, 4, 2048, 4096
T = 64
N_CORES = 8
EPC = E // N_CORES          # experts per core = 2
KT = H // 128               # 16 k-tiles of 128 over H
FCH = 8                     # f-chunks of 512 over F
FC = F // FCH               # 512
NCH = EPC * FCH             # 16 weight chunks per core per matrix


def _ensure_axon_hooks():
    """antenv.axon_hooks is missing from the stub antenv shipped in some
    containers; run_bass_kernel_spmd(trace=True) imports it under axon.
    Register the ctypes NTFF hook when libaxon_pjrt.so is present, else a
    None-returning stub so tracing degrades instead of crashing."""
    try:
        import antenv.axon_hooks  # noqa: F401
        return
    except ImportError:
        pass
    try:
        import antenv
    except ImportError:
        return
    mod = types.ModuleType("antenv.axon_hooks")
    _hook = [None]
    mod.set_axon_ntff_profile_hook = lambda h: _hook.__setitem__(0, h)
    mod.get_axon_ntff_profile_hook = lambda: _hook[0]
    sys.modules["antenv.axon_hooks"] = mod
    antenv.axon_hooks = mod
    try:
        from trn_agent_boot.trn_boot import _ntff_profile_via_ctypes

        so_path = "/opt/axon/libaxon_pjrt.so"
        if os.path.exists(so_path):
            h = _ntff_profile_via_ctypes(so_path)
            if h is not None:
                mod.set_axon_ntff_profile_hook(h)
    except Exception:
        pass


def _build_nc():
    import concourse.mybir as mybir
    import concourse.tile as tile
    from concourse import bacc

    f32 = mybir.dt.float32
    bf16 = mybir.dt.bfloat16

    nc = bacc.Bacc("TRN2", debug=False, num_devices=N_CORES)
    xt_d = nc.dram_tensor("xt", [128, KT * T], bf16, kind="ExternalInput")
    w1_d = nc.dram_tensor("w1t", [NCH, 128, KT * FC], bf16, kind="ExternalInput")
    v1_d = nc.dram_tensor("v1t", [NCH, 128, KT * FC], bf16, kind="ExternalInput")
    w2_d = nc.dram_tensor("w2s", [NCH, 128, 4 * H], bf16, kind="ExternalInput")
    r_d = nc.dram_tensor("r", [T, EPC], f32, kind="ExternalInput")
    out_d = nc.dram_tensor("out", [T, H], f32, kind="ExternalOutput")

    act = mybir.ActivationFunctionType

    with tile.TileContext(nc) as tc:
        with (
            tc.tile_pool(name="const", bufs=1) as const_pool,
            tc.tile_pool(name="w1", bufs=3) as w1_pool,
            tc.tile_pool(name="v1", bufs=3) as v1_pool,
            tc.tile_pool(name="w2", bufs=3) as w2_pool,
            tc.tile_pool(name="acts", bufs=4) as acts_pool,
            tc.tile_pool(name="ps_gate", bufs=2, space="PSUM") as ps_gate,
            tc.tile_pool(name="ps_up", bufs=2, space="PSUM") as ps_up,
            tc.tile_pool(name="ps_tp", bufs=2, space="PSUM") as ps_tp,
            tc.tile_pool(name="ps_down", bufs=1, space="PSUM") as ps_down,
        ):
            # constants / whole-kernel tiles (scalar HWDGE queue, so they
            # don't queue behind the weight stream on the sync queue)
            xt_sb = const_pool.tile([128, KT * T], bf16)
            nc.scalar.dma_start(xt_sb[:], xt_d[:])
            r_sb = const_pool.tile([T, EPC], f32)
            nc.scalar.dma_start(r_sb[:], r_d[:])
            ident = const_pool.tile([64, 64], bf16)
            from concourse.masks import make_identity

            make_identity(nc, ident)

            # persistent down-projection accumulator:
            # [0:64, 0:1024] = hid 0..1023, [64:128, 0:1024] = hid 1024..2047
            down_ps = ps_down.tile([128, 1024], mybir.dt.float32)

            HKT = KT // 2  # k-tiles per half-chunk DMA

            def piece(e, w1a, w1b, v1a, v1b, w2c, fo, fw, first, last):
                """Process f-range [fo, fo+fw) of the current 512-wide chunk."""
                gate_ps = ps_gate.tile([T, fw], mybir.dt.float32, tag="gate")
                up_ps = ps_up.tile([T, fw], mybir.dt.float32, tag="up")
                for i in range(KT):
                    wsrc = w1a if i < HKT else w1b
                    lo = (i % HKT) * FC + fo
                    nc.tensor.matmul(
                        gate_ps[:],
                        xt_sb[:, i * T : (i + 1) * T],
                        wsrc[:, lo : lo + fw],
                        start=(i == 0),
                        stop=(i == KT - 1),
                    )
                for i in range(KT):
                    vsrc = v1a if i < HKT else v1b
                    lo = (i % HKT) * FC + fo
                    nc.tensor.matmul(
                        up_ps[:],
                        xt_sb[:, i * T : (i + 1) * T],
                        vsrc[:, lo : lo + fw],
                        start=(i == 0),
                        stop=(i == KT - 1),
                    )

                gate_s = acts_pool.tile([T, fw], bf16, tag="gate_s")
                nc.scalar.activation(gate_s[:], gate_ps[:], act.Silu)
                up_s = acts_pool.tile([T, fw], bf16, tag="up_s")
                nc.scalar.activation(
                    up_s[:], up_ps[:], act.Copy, scale=r_sb[:, e : e + 1]
                )
                h = acts_pool.tile([T, fw], bf16, tag="h")
                nc.vector.tensor_mul(h[:], gate_s[:], up_s[:])

                # transpose h [64, fw] -> hT tiles [128, 64] via PE
                ntp = fw // 128
                tp_ps = ps_tp.tile([128, ntp * T], bf16, tag="tp")
                for j in range(ntp):
                    nc.tensor.transpose(
                        tp_ps[:, j * T : (j + 1) * T],
                        h[:, j * 128 : (j + 1) * 128],
                        ident[:],
                    )
                hT = acts_pool.tile([128, ntp * T], bf16, tag="hT")
                nc.vector.tensor_copy(hT[:], tp_ps[:])

                for j in range(ntp):
                    jg = (fo + j * 128) // 128  # f-tile index within chunk
                    for q in range(4):
                        if q < 2:
                            dst = down_ps[0:T, q * 512 : (q + 1) * 512]
                        else:
                            dst = down_ps[64 : 64 + T, (q - 2) * 512 : (q - 1) * 512]
                        nc.tensor.matmul(
                            dst,
                            hT[:, j * T : (j + 1) * T],
                            w2c[:, jg * H + q * 512 : jg * H + (q + 1) * 512],
                            start=(first and j == 0),
                            stop=(last and j == ntp - 1),
                        )

            for e in range(EPC):
                for c in range(FCH):
                    ci = e * FCH + c
                    # half-split weight tiles: PE can start on half A while
                    # half B is still in flight. w1/w2 issue on the sync
                    # HWDGE queue, v1 on the scalar queue (parallel rings).
                    w1a = w1_pool.tile([128, HKT * FC], bf16, tag="w1a")
                    nc.sync.dma_start(w1a[:], w1_d[ci, :, : HKT * FC])
                    w1b = w1_pool.tile([128, HKT * FC], bf16, tag="w1b")
                    nc.sync.dma_start(w1b[:], w1_d[ci, :, HKT * FC :])
                    v1a = v1_pool.tile([128, HKT * FC], bf16, tag="v1a")
                    nc.scalar.dma_start(v1a[:], v1_d[ci, :, : HKT * FC])
                    v1b = v1_pool.tile([128, HKT * FC], bf16, tag="v1b")
                    nc.scalar.dma_start(v1b[:], v1_d[ci, :, HKT * FC :])
                    w2c = w2_pool.tile([128, 4 * H], bf16, tag="w2c")
                    nc.gpsimd.dma_start(w2c[:], w2_d[ci])

                    glast = e == EPC - 1 and c == FCH - 1
                    first = e == 0 and c == 0
                    if glast:
                        # split the final chunk into 128-wide pieces to
                        # shorten the end-of-kernel dependency chain
                        for s in range(4):
                            piece(
                                e, w1a, w1b, v1a, v1b, w2c,
                                s * 128, 128,
                                first=False, last=(s == 3),
                            )
                    else:
                        piece(e, w1a, w1b, v1a, v1b, w2c, 0, FC, first, False)

            out_sb = const_pool.tile([128, 1024], mybir.dt.float32)
            nc.vector.tensor_copy(out_sb[0:T], down_ps[0:T])
            nc.vector.tensor_copy(out_sb[64 : 64 + T], down_ps[64 : 64 + T])
            nc.sync.dma_start(out_d[:, 0:1024], out_sb[0:T])
            nc.sync.dma_start(out_d[:, 1024:2048], out_sb[64 : 64 + T])

    nc.compile()
    return nc


_NC_CACHE = None


def _get_nc():
    global _NC_CACHE
    if _NC_CACHE is None:
        _NC_CACHE = _build_nc()
    return _NC_CACHE


def _swizzle_ffn(wt):
    """[H, F] (h, f) -> [FCH, 128, KT*FC] so chunk c is a contiguous
    [128, 8192] block with [p, i*FC + f] = wt[i*128 + p, c*FC + f]."""
    a = wt.reshape(KT, 128, FCH, FC)          # (i, p, c, f)
    return np.ascontiguousarray(a.transpose(2, 1, 0, 3)).reshape(FCH, 128, KT * FC)


def _swizzle_down(w2e):
    """[F, H] (f, hid) -> [FCH, 128, 4*H] so chunk c is contiguous
    [128, 8192] with [p, j*H + hid] = w2e[c*FC + j*128 + p, hid]."""
    a = w2e.reshape(FCH, 4, 128, H)           # (c, j, p, hid)
    return np.ascontiguousarray(a.transpose(0, 2, 1, 3)).reshape(FCH, 128, 4 * H)


def kernel(x, weights, top_weights, top_experts, w1, v1, w2):
    _ensure_axon_hooks()
    from concourse.bass_utils import run_bass_kernel_spmd

    x = np.asarray(x, dtype=np.float32).reshape(T, H)
    top_weights = np.asarray(top_weights, dtype=np.float32)
    top_experts = np.asarray(top_experts).astype(np.int64)
    w1 = np.asarray(w1, dtype=np.float32).reshape(E, F, H)
    v1 = np.asarray(v1, dtype=np.float32).reshape(E, F, H)
    w2 = np.asarray(w2, dtype=np.float32).reshape(E, F, H)

    # dense routing weights [T, E] (scatter-ADD: duplicate experts sum)
    r = np.zeros((T, E), np.float32)
    np.add.at(r, (np.arange(T)[:, None], top_experts), top_weights)

    # x transposed/swizzled: [128, KT*T] with [p, i*T + t] = x[t, i*128 + p]
    xt = np.ascontiguousarray(
        x.T.reshape(KT, 128, T).transpose(1, 0, 2)
    ).reshape(128, KT * T).astype(BF16)

    in_maps = []
    for core in range(N_CORES):
        es = [core * EPC + k for k in range(EPC)]
        w1t = np.concatenate(
            [_swizzle_ffn(w1[e].T.astype(BF16)) for e in es], axis=0
        )
        v1t = np.concatenate(
            [_swizzle_ffn(v1[e].T.astype(BF16)) for e in es], axis=0
        )
        w2s = np.concatenate(
            [_swizzle_down(w2[e].astype(BF16)) for e in es], axis=0
        )
        in_maps.append(
            {
                "xt": xt,
                "w1t": w1t,
                "v1t": v1t,
                "w2s": w2s,
                "r": np.ascontiguousarray(r[:, es]),
            }
        )

    nc = _get_nc()
    res = run_bass_kernel_spmd(nc, in_maps, core_ids=list(range(N_CORES)))
    out = np.zeros((T, H), np.float32)
    for c in range(N_CORES):
        out += res.results[c]["out"]
    return out.reshape(64, 1, H)


# revision 7
# speedup vs baseline: 1.0585x; 1.0585x over previous
"""DbrxExperts MoE kernel for 8 Trainium2 NeuronCores (expert-parallel).

Problem: E=16 experts, top_k=4, H=2048, F=4096, T=64 tokens.
out = sum_e r[:, e] * (silu(x @ w1_e.T) * (x @ v1_e.T)) @ w2_e
with r = scatter-add of top_weights into dense [T, E].

Strategy: expert-parallel across 8 cores (2 experts per core). Each core
streams its 2 experts' weights (bf16-cast on host: halves HBM traffic;
fp32 PSUM accumulation keeps rel-err ~4e-3) and computes a partial
output [T, H]; host sums the 8 partials. Routing weights are folded into
the `up` projection drain, so the down-projection accumulates both local
experts directly in PSUM.

Weight layouts are pre-swizzled on the host so every weight DMA is a
fully contiguous 2 MiB transfer of shape [128, 8192] bf16.
"""

import os
import sys
import types

import numpy as np
import ml_dtypes

BF16 = ml_dtypes.bfloat16

E, TOPK, H, F = 16, 4, 2048, 4096
T = 64
N_CORES = 8
EPC = E // N_CORES          # experts per core = 2
KT = H // 128               # 16 k-tiles of 128 over H
FCH = 8                     # f-chunks of 512 over F
FC = F // FCH               # 512
NCH = EPC * FCH             # 16 weight chunks per core per matrix


def _ensure_axon_hooks():
    """antenv.axon_hooks is missing from the stub antenv shipped in some
    containers; run_bass_kernel_spmd(trace=True) imports it under axon.
    Register the ctypes NTFF hook when libaxon_pjrt.so is present, else a
    None-returning stub so tracing degrades instead of crashing."""
    try:
        import antenv.axon_hooks  # noqa: F401
        return
    except ImportError:
        pass
    try:
        import antenv
    except ImportError:
        return
    mod = types.ModuleType("antenv.axon_hooks")
    _hook = [None]
    mod.set_axon_ntff_profile_hook = lambda h: _hook.__setitem__(0, h)
    mod.get_axon_ntff_profile_hook = lambda: _hook[0]
    sys.modules["antenv.axon_hooks"] = mod
    antenv.axon_hooks = mod
    try:
        from trn_agent_boot.trn_boot import _ntff_profile_via_ctypes

        so_path = "/opt/axon/libaxon_pjrt.so"
        if os.path.exists(so_path):
            h = _ntff_profile_via_ctypes(so_path)
            if h is not None:
                mod.set_axon_ntff_profile_hook(h)
    except Exception:
        pass


def _build_nc():
    import concourse.mybir as mybir
    import concourse.tile as tile
    from concourse import bacc

    f32 = mybir.dt.float32
    bf16 = mybir.dt.bfloat16

    nc = bacc.Bacc("TRN2", debug=False, num_devices=N_CORES)
    xt_d = nc.dram_tensor("xt", [128, KT * T], bf16, kind="ExternalInput")
    w1_d = nc.dram_tensor("w1t", [NCH, 128, KT * FC], bf16, kind="ExternalInput")
    v1_d = nc.dram_tensor("v1t", [NCH, 128, KT * FC], bf16, kind="ExternalInput")
    w2_d = nc.dram_tensor("w2s", [NCH, 128, 4 * H], bf16, kind="ExternalInput")
    r_d = nc.dram_tensor("r", [T, EPC], f32, kind="ExternalInput")
    out_d = nc.dram_tensor("out", [T, H], f32, kind="ExternalOutput")

    act = mybir.ActivationFunctionType

    with tile.TileContext(nc) as tc:
        with (
            tc.tile_pool(name="const", bufs=1) as const_pool,
            tc.tile_pool(name="w1", bufs=3) as w1_pool,
            tc.tile_pool(name="v1", bufs=3) as v1_pool,
            tc.tile_pool(name="w2", bufs=4) as w2_pool,
            tc.tile_pool(name="acts", bufs=4) as acts_pool,
            tc.tile_pool(name="ps_gate", bufs=2, space="PSUM") as ps_gate,
            tc.tile_pool(name="ps_up", bufs=2, space="PSUM") as ps_up,
            tc.tile_pool(name="ps_tp", bufs=2, space="PSUM") as ps_tp,
            tc.tile_pool(name="ps_down", bufs=1, space="PSUM") as ps_down,
        ):
            # constants / whole-kernel tiles (scalar HWDGE queue, so they
            # don't queue behind the weight stream on the sync queue)
            xt_sb = const_pool.tile([128, KT * T], bf16)
            nc.scalar.dma_start(xt_sb[:], xt_d[:])
            r_sb = const_pool.tile([T, EPC], f32)
            nc.scalar.dma_start(r_sb[:], r_d[:])
            ident = const_pool.tile([64, 64], bf16)
            from concourse.masks import make_identity

            make_identity(nc, ident)

            # persistent down-projection accumulator:
            # [0:64, 0:1024] = hid 0..1023, [64:128, 0:1024] = hid 1024..2047
            down_ps = ps_down.tile([128, 1024], mybir.dt.float32)

            HKT = KT // 2  # k-tiles per half-chunk DMA

            def piece(e, w1a, w1b, v1a, v1b, w2c, fo, fw, first, last):
                """Process f-range [fo, fo+fw) of the current 512-wide chunk."""
                gate_ps = ps_gate.tile([T, fw], mybir.dt.float32, tag="gate")
                up_ps = ps_up.tile([T, fw], mybir.dt.float32, tag="up")
                for i in range(KT):
                    wsrc = w1a if i < HKT else w1b
                    lo = (i % HKT) * FC + fo
                    nc.tensor.matmul(
                        gate_ps[:],
                        xt_sb[:, i * T : (i + 1) * T],
                        wsrc[:, lo : lo + fw],
                        start=(i == 0),
                        stop=(i == KT - 1),
                    )
                for i in range(KT):
                    vsrc = v1a if i < HKT else v1b
                    lo = (i % HKT) * FC + fo
                    nc.tensor.matmul(
                        up_ps[:],
                        xt_sb[:, i * T : (i + 1) * T],
                        vsrc[:, lo : lo + fw],
                        start=(i == 0),
                        stop=(i == KT - 1),
                    )

                gate_s = acts_pool.tile([T, fw], bf16, tag="gate_s")
                nc.scalar.activation(gate_s[:], gate_ps[:], act.Silu)
                up_s = acts_pool.tile([T, fw], bf16, tag="up_s")
                nc.scalar.activation(
                    up_s[:], up_ps[:], act.Copy, scale=r_sb[:, e : e + 1]
                )
                h = acts_pool.tile([T, fw], bf16, tag="h")
                nc.vector.tensor_mul(h[:], gate_s[:], up_s[:])

                # transpose h [64, fw] -> hT tiles [128, 64] via PE
                ntp = fw // 128
                tp_ps = ps_tp.tile([128, ntp * T], bf16, tag="tp")
                for j in range(ntp):
                    nc.tensor.transpose(
                        tp_ps[:, j * T : (j + 1) * T],
                        h[:, j * 128 : (j + 1) * 128],
                        ident[:],
                    )
                hT = acts_pool.tile([128, ntp * T], bf16, tag="hT")
                nc.vector.tensor_copy(hT[:], tp_ps[:])

                for j in range(ntp):
                    jg = (fo + j * 128) // 128  # f-tile index within chunk
                    for q in range(4):
                        if q < 2:
                            dst = down_ps[0:T, q * 512 : (q + 1) * 512]
                        else:
                            dst = down_ps[64 : 64 + T, (q - 2) * 512 : (q - 1) * 512]
                        nc.tensor.matmul(
                            dst,
                            hT[:, j * T : (j + 1) * T],
                            w2c[:, jg * H + q * 512 : jg * H + (q + 1) * 512],
                            start=(first and j == 0),
                            stop=(last and j == ntp - 1),
                        )

            for e in range(EPC):
                for c in range(FCH):
                    ci = e * FCH + c
                    # half-split weight tiles: PE can start on half A while
                    # half B is still in flight. w1/w2 issue on the sync
                    # HWDGE queue, v1 on the scalar queue (parallel rings).
                    w1a = w1_pool.tile([128, HKT * FC], bf16, tag="w1a")
                    nc.sync.dma_start(w1a[:], w1_d[ci, :, : HKT * FC])
                    w1b = w1_pool.tile([128, HKT * FC], bf16, tag="w1b")
                    nc.sync.dma_start(w1b[:], w1_d[ci, :, HKT * FC :])
                    v1a = v1_pool.tile([128, HKT * FC], bf16, tag="v1a")
                    nc.scalar.dma_start(v1a[:], v1_d[ci, :, : HKT * FC])
                    v1b = v1_pool.tile([128, HKT * FC], bf16, tag="v1b")
                    nc.scalar.dma_start(v1b[:], v1_d[ci, :, HKT * FC :])
                    w2c = w2_pool.tile([128, 4 * H], bf16, tag="w2c")
                    nc.scalar.dma_start(w2c[:], w2_d[ci])

                    glast = e == EPC - 1 and c == FCH - 1
                    first = e == 0 and c == 0
                    if glast:
                        # split the final chunk into 128-wide pieces to
                        # shorten the end-of-kernel dependency chain
                        for s in range(4):
                            piece(
                                e, w1a, w1b, v1a, v1b, w2c,
                                s * 128, 128,
                                first=False, last=(s == 3),
                            )
                    else:
                        piece(e, w1a, w1b, v1a, v1b, w2c, 0, FC, first, False)

            out_sb = const_pool.tile([128, 1024], mybir.dt.float32)
            nc.vector.tensor_copy(out_sb[0:T], down_ps[0:T])
            nc.vector.tensor_copy(out_sb[64 : 64 + T], down_ps[64 : 64 + T])
            nc.sync.dma_start(out_d[:, 0:1024], out_sb[0:T])
            nc.sync.dma_start(out_d[:, 1024:2048], out_sb[64 : 64 + T])

    nc.compile()
    return nc


_NC_CACHE = None


def _get_nc():
    global _NC_CACHE
    if _NC_CACHE is None:
        _NC_CACHE = _build_nc()
    return _NC_CACHE


def _swizzle_ffn(wt):
    """[H, F] (h, f) -> [FCH, 128, KT*FC] so chunk c is a contiguous
    [128, 8192] block with [p, i*FC + f] = wt[i*128 + p, c*FC + f]."""
    a = wt.reshape(KT, 128, FCH, FC)          # (i, p, c, f)
    return np.ascontiguousarray(a.transpose(2, 1, 0, 3)).reshape(FCH, 128, KT * FC)


def _swizzle_down(w2e):
    """[F, H] (f, hid) -> [FCH, 128, 4*H] so chunk c is contiguous
    [128, 8192] with [p, j*H + hid] = w2e[c*FC + j*128 + p, hid]."""
    a = w2e.reshape(FCH, 4, 128, H)           # (c, j, p, hid)
    return np.ascontiguousarray(a.transpose(0, 2, 1, 3)).reshape(FCH, 128, 4 * H)


def kernel(x, weights, top_weights, top_experts, w1, v1, w2):
    _ensure_axon_hooks()
    from concourse.bass_utils import run_bass_kernel_spmd

    x = np.asarray(x, dtype=np.float32).reshape(T, H)
    top_weights = np.asarray(top_weights, dtype=np.float32)
    top_experts = np.asarray(top_experts).astype(np.int64)
    w1 = np.asarray(w1, dtype=np.float32).reshape(E, F, H)
    v1 = np.asarray(v1, dtype=np.float32).reshape(E, F, H)
    w2 = np.asarray(w2, dtype=np.float32).reshape(E, F, H)

    # dense routing weights [T, E] (scatter-ADD: duplicate experts sum)
    r = np.zeros((T, E), np.float32)
    np.add.at(r, (np.arange(T)[:, None], top_experts), top_weights)

    # x transposed/swizzled: [128, KT*T] with [p, i*T + t] = x[t, i*128 + p]
    xt = np.ascontiguousarray(
        x.T.reshape(KT, 128, T).transpose(1, 0, 2)
    ).reshape(128, KT * T).astype(BF16)

    in_maps = []
    for core in range(N_CORES):
        es = [core * EPC + k for k in range(EPC)]
        w1t = np.concatenate(
            [_swizzle_ffn(w1[e].T.astype(BF16)) for e in es], axis=0
        )
        v1t = np.concatenate(
            [_swizzle_ffn(v1[e].T.astype(BF16)) for e in es], axis=0
        )
        w2s = np.concatenate(
            [_swizzle_down(w2[e].astype(BF16)) for e in es], axis=0
        )
        in_maps.append(
            {
                "xt": xt,
                "w1t": w1t,
                "v1t": v1t,
                "w2s": w2s,
                "r": np.ascontiguousarray(r[:, es]),
            }
        )

    nc = _get_nc()
    res = run_bass_kernel_spmd(nc, in_maps, core_ids=list(range(N_CORES)))
    out = np.zeros((T, H), np.float32)
    for c in range(N_CORES):
        out += res.results[c]["out"]
    return out.reshape(64, 1, H)


# revision 9
# speedup vs baseline: 1.2602x; 1.1906x over previous
"""DbrxExperts MoE kernel for 8 Trainium2 NeuronCores (expert-parallel).

Problem: E=16 experts, top_k=4, H=2048, F=4096, T=64 tokens.
out = sum_e r[:, e] * (silu(x @ w1_e.T) * (x @ v1_e.T)) @ w2_e
with r = scatter-add of top_weights into dense [T, E].

Strategy: expert-parallel across 8 cores (2 experts per core). Each core
streams its 2 experts' weights (bf16-cast on host: halves HBM traffic;
fp32 PSUM accumulation keeps rel-err ~4e-3) and computes a partial
output [T, H]; host sums the 8 partials. Routing weights are folded into
the `up` projection drain, so the down-projection accumulates both local
experts directly in PSUM.

Weight layouts are pre-swizzled on the host so every weight DMA is a
fully contiguous 2 MiB transfer of shape [128, 8192] bf16.
"""

import os
import sys
import types

import numpy as np
import ml_dtypes

BF16 = ml_dtypes.bfloat16

E, TOPK, H, F = 16, 4, 2048, 4096
T = 64
N_CORES = 8
EPC = E // N_CORES          # experts per core = 2
KT = H // 128               # 16 k-tiles of 128 over H
FCH = 8                     # f-chunks of 512 over F
FC = F // FCH               # 512
NCH = EPC * FCH             # 16 weight chunks per core per matrix


def _ensure_axon_hooks():
    """antenv.axon_hooks is missing from the stub antenv shipped in some
    containers; run_bass_kernel_spmd(trace=True) imports it under axon.
    Register the ctypes NTFF hook when libaxon_pjrt.so is present, else a
    None-returning stub so tracing degrades instead of crashing."""
    try:
        import antenv.axon_hooks  # noqa: F401
        return
    except ImportError:
        pass
    try:
        import antenv
    except ImportError:
        return
    mod = types.ModuleType("antenv.axon_hooks")
    _hook = [None]
    mod.set_axon_ntff_profile_hook = lambda h: _hook.__setitem__(0, h)
    mod.get_axon_ntff_profile_hook = lambda: _hook[0]
    sys.modules["antenv.axon_hooks"] = mod
    antenv.axon_hooks = mod
    try:
        from trn_agent_boot.trn_boot import _ntff_profile_via_ctypes

        so_path = "/opt/axon/libaxon_pjrt.so"
        if os.path.exists(so_path):
            h = _ntff_profile_via_ctypes(so_path)
            if h is not None:
                mod.set_axon_ntff_profile_hook(h)
    except Exception:
        pass


def _build_nc():
    import concourse.mybir as mybir
    import concourse.tile as tile
    from concourse import bacc

    f32 = mybir.dt.float32
    bf16 = mybir.dt.bfloat16

    nc = bacc.Bacc("TRN2", debug=False, num_devices=N_CORES)
    xt_d = nc.dram_tensor("xt", [128, KT * T], bf16, kind="ExternalInput")
    w1_d = nc.dram_tensor("w1t", [NCH, 128, KT * FC], bf16, kind="ExternalInput")
    v1_d = nc.dram_tensor("v1t", [NCH, 128, KT * FC], bf16, kind="ExternalInput")
    w2_d = nc.dram_tensor("w2s", [NCH, 128, 4 * H], bf16, kind="ExternalInput")
    r_d = nc.dram_tensor("r", [T, EPC], f32, kind="ExternalInput")
    out_d = nc.dram_tensor("out", [T, H], f32, kind="ExternalOutput")

    act = mybir.ActivationFunctionType

    with tile.TileContext(nc) as tc:
        with (
            tc.tile_pool(name="const", bufs=1) as const_pool,
            tc.tile_pool(name="w1", bufs=3) as w1_pool,
            tc.tile_pool(name="v1", bufs=3) as v1_pool,
            tc.tile_pool(name="w2", bufs=4) as w2_pool,
            tc.tile_pool(name="acts", bufs=4) as acts_pool,
            tc.tile_pool(name="ps_gate", bufs=2, space="PSUM") as ps_gate,
            tc.tile_pool(name="ps_up", bufs=2, space="PSUM") as ps_up,
            tc.tile_pool(name="ps_tp", bufs=2, space="PSUM") as ps_tp,
            tc.tile_pool(name="ps_down", bufs=1, space="PSUM") as ps_down,
        ):
            # constants / whole-kernel tiles (scalar HWDGE queue, so they
            # don't queue behind the weight stream on the sync queue)
            xt_sb = const_pool.tile([128, KT * T], bf16)
            nc.scalar.dma_start(xt_sb[:], xt_d[:])
            r_sb = const_pool.tile([T, EPC], f32)
            nc.scalar.dma_start(r_sb[:], r_d[:])
            ident = const_pool.tile([64, 64], bf16)
            from concourse.masks import make_identity

            make_identity(nc, ident)

            # persistent down-projection accumulator:
            # [0:64, 0:1024] = hid 0..1023, [64:128, 0:1024] = hid 1024..2047
            down_ps = ps_down.tile([128, 1024], mybir.dt.float32)

            HKT = KT // 2  # k-tiles per half-chunk DMA

            def piece(e, w1a, w1b, v1a, v1b, w2c, fo, fw, first, last):
                """Process f-range [fo, fo+fw) of the current 512-wide chunk."""
                gate_ps = ps_gate.tile([T, fw], mybir.dt.float32, tag="gate")
                up_ps = ps_up.tile([T, fw], mybir.dt.float32, tag="up")
                for i in range(KT):
                    wsrc = w1a if i < HKT else w1b
                    lo = (i % HKT) * FC + fo
                    nc.tensor.matmul(
                        gate_ps[:],
                        xt_sb[:, i * T : (i + 1) * T],
                        wsrc[:, lo : lo + fw],
                        start=(i == 0),
                        stop=(i == KT - 1),
                    )
                for i in range(KT):
                    vsrc = v1a if i < HKT else v1b
                    lo = (i % HKT) * FC + fo
                    nc.tensor.matmul(
                        up_ps[:],
                        xt_sb[:, i * T : (i + 1) * T],
                        vsrc[:, lo : lo + fw],
                        start=(i == 0),
                        stop=(i == KT - 1),
                    )

                gate_s = acts_pool.tile([T, fw], bf16, tag="gate_s")
                nc.scalar.activation(gate_s[:], gate_ps[:], act.Silu)
                up_s = acts_pool.tile([T, fw], bf16, tag="up_s")
                nc.vector.tensor_scalar_mul(up_s[:], up_ps[:], r_sb[:, e : e + 1])
                h = acts_pool.tile([T, fw], bf16, tag="h")
                nc.vector.tensor_mul(h[:], gate_s[:], up_s[:])

                # transpose h [64, fw] -> hT tiles [128, 64] via PE
                ntp = fw // 128
                tp_ps = ps_tp.tile([128, ntp * T], bf16, tag="tp")
                for j in range(ntp):
                    nc.tensor.transpose(
                        tp_ps[:, j * T : (j + 1) * T],
                        h[:, j * 128 : (j + 1) * 128],
                        ident[:],
                    )
                hT = acts_pool.tile([128, ntp * T], bf16, tag="hT")
                nc.vector.tensor_copy(hT[:], tp_ps[:])

                for j in range(ntp):
                    jg = (fo + j * 128) // 128  # f-tile index within chunk
                    for q in range(4):
                        if q < 2:
                            dst = down_ps[0:T, q * 512 : (q + 1) * 512]
                        else:
                            dst = down_ps[64 : 64 + T, (q - 2) * 512 : (q - 1) * 512]
                        nc.tensor.matmul(
                            dst,
                            hT[:, j * T : (j + 1) * T],
                            w2c[:, jg * H + q * 512 : jg * H + (q + 1) * 512],
                            start=(first and j == 0),
                            stop=(last and j == ntp - 1),
                        )

            for e in range(EPC):
                for c in range(FCH):
                    ci = e * FCH + c
                    # half-split weight tiles: PE can start on half A while
                    # half B is still in flight. w1/w2 issue on the sync
                    # HWDGE queue, v1 on the scalar queue (parallel rings).
                    w1a = w1_pool.tile([128, HKT * FC], bf16, tag="w1a")
                    nc.sync.dma_start(w1a[:], w1_d[ci, :, : HKT * FC])
                    w1b = w1_pool.tile([128, HKT * FC], bf16, tag="w1b")
                    nc.sync.dma_start(w1b[:], w1_d[ci, :, HKT * FC :])
                    v1a = v1_pool.tile([128, HKT * FC], bf16, tag="v1a")
                    nc.scalar.dma_start(v1a[:], v1_d[ci, :, : HKT * FC])
                    v1b = v1_pool.tile([128, HKT * FC], bf16, tag="v1b")
                    nc.scalar.dma_start(v1b[:], v1_d[ci, :, HKT * FC :])
                    w2c = w2_pool.tile([128, 4 * H], bf16, tag="w2c")
                    nc.sync.dma_start(w2c[:], w2_d[ci])

                    glast = e == EPC - 1 and c == FCH - 1
                    first = e == 0 and c == 0
                    if glast:
                        # split the final chunk into 128-wide pieces to
                        # shorten the end-of-kernel dependency chain
                        for s in range(4):
                            piece(
                                e, w1a, w1b, v1a, v1b, w2c,
                                s * 128, 128,
                                first=False, last=(s == 3),
                            )
                    else:
                        piece(e, w1a, w1b, v1a, v1b, w2c, 0, FC, first, False)

            out_sb = const_pool.tile([128, 1024], mybir.dt.float32)
            nc.vector.tensor_copy(out_sb[0:T], down_ps[0:T])
            nc.vector.tensor_copy(out_sb[64 : 64 + T], down_ps[64 : 64 + T])
            nc.sync.dma_start(out_d[:, 0:1024], out_sb[0:T])
            nc.sync.dma_start(out_d[:, 1024:2048], out_sb[64 : 64 + T])

    nc.compile()
    return nc


_NC_CACHE = None


def _get_nc():
    global _NC_CACHE
    if _NC_CACHE is None:
        _NC_CACHE = _build_nc()
    return _NC_CACHE


def _swizzle_ffn(wt):
    """[H, F] (h, f) -> [FCH, 128, KT*FC] so chunk c is a contiguous
    [128, 8192] block with [p, i*FC + f] = wt[i*128 + p, c*FC + f]."""
    a = wt.reshape(KT, 128, FCH, FC)          # (i, p, c, f)
    return np.ascontiguousarray(a.transpose(2, 1, 0, 3)).reshape(FCH, 128, KT * FC)


def _swizzle_down(w2e):
    """[F, H] (f, hid) -> [FCH, 128, 4*H] so chunk c is contiguous
    [128, 8192] with [p, j*H + hid] = w2e[c*FC + j*128 + p, hid]."""
    a = w2e.reshape(FCH, 4, 128, H)           # (c, j, p, hid)
    return np.ascontiguousarray(a.transpose(0, 2, 1, 3)).reshape(FCH, 128, 4 * H)


def kernel(x, weights, top_weights, top_experts, w1, v1, w2):
    _ensure_axon_hooks()
    from concourse.bass_utils import run_bass_kernel_spmd

    x = np.asarray(x, dtype=np.float32).reshape(T, H)
    top_weights = np.asarray(top_weights, dtype=np.float32)
    top_experts = np.asarray(top_experts).astype(np.int64)
    w1 = np.asarray(w1, dtype=np.float32).reshape(E, F, H)
    v1 = np.asarray(v1, dtype=np.float32).reshape(E, F, H)
    w2 = np.asarray(w2, dtype=np.float32).reshape(E, F, H)

    # dense routing weights [T, E] (scatter-ADD: duplicate experts sum)
    r = np.zeros((T, E), np.float32)
    np.add.at(r, (np.arange(T)[:, None], top_experts), top_weights)

    # x transposed/swizzled: [128, KT*T] with [p, i*T + t] = x[t, i*128 + p]
    xt = np.ascontiguousarray(
        x.T.reshape(KT, 128, T).transpose(1, 0, 2)
    ).reshape(128, KT * T).astype(BF16)

    in_maps = []
    for core in range(N_CORES):
        es = [core * EPC + k for k in range(EPC)]
        w1t = np.concatenate(
            [_swizzle_ffn(w1[e].T.astype(BF16)) for e in es], axis=0
        )
        v1t = np.concatenate(
            [_swizzle_ffn(v1[e].T.astype(BF16)) for e in es], axis=0
        )
        w2s = np.concatenate(
            [_swizzle_down(w2[e].astype(BF16)) for e in es], axis=0
        )
        in_maps.append(
            {
                "xt": xt,
                "w1t": w1t,
                "v1t": v1t,
                "w2s": w2s,
                "r": np.ascontiguousarray(r[:, es]),
            }
        )

    nc = _get_nc()
    res = run_bass_kernel_spmd(nc, in_maps, core_ids=list(range(N_CORES)))
    out = np.zeros((T, H), np.float32)
    for c in range(N_CORES):
        out += res.results[c]["out"]
    return out.reshape(64, 1, H)
